# revision 4
# baseline (speedup 1.0000x reference)
"""Llama GQA attention block (B=1, S=2048, D=4096, 32 Q heads / 8 KV heads,
hd=128) on 8 trn2 NeuronCores.

Sharding: tensor-parallel by attention head. Core c owns q-heads 4c..4c+3 and
kv-head c (exactly one GQA group), computing QKV projection for its rows of
wqkv, RoPE, attention, then an AllGather of the per-core attention outputs and
a 512-column shard of the output projection. Host concatenates the 8 column
shards.

Layout trick: everything is kept "transposed" (feature on the partition axis)
so the PE contractions never need an on-device transpose of activations:
  - x, wqkv, wo, mask arrive host-pre-transposed.
  - scores are computed directly as S^T = [sk, sq] tiles (lhsT=k^T, rhs=q^T).
  - softmax skips the max-subtraction (logits are O(5) for this problem:
    scores scaled by 1/sqrt(128), plus additive mask), exp on ACT engine,
    denominators via a ones-vector PE reduction and a rank-1 broadcast matmul.
  - attention output accumulates as y^T = [hd, sq] (lhsT=v, rhs=P^T).
RoPE's rotate-half is a cross-partition half-swap, done with 4 quadrant
stream_shuffle copies plus host-prepared sign-folded sin tables.

Matmuls run in bf16 (fp32 PSUM accumulate); fp32 matmul on trn2 is 4x slower.
The kernel is specialized at build time to the observed mask structure:
fully-masked [sk=128, sq=512] tiles are skipped, all-zero tiles skip the
mask-add (handles both the causal mask and an all-zeros mask exactly).
"""

import math

import numpy as np

DIM = 4096
S = 2048
HD = 128
QH = 4          # q heads per core
NCORES = 8
E_TILES = 6     # 4 q + 1 k + 1 v row-tiles of the per-core qkv projection
KT = DIM // 128  # 32 contraction tiles
NSB = 4         # sq blocks
SBW = 512       # sq block width
SKT = S // 128  # 16 sk tiles
EC = 512        # output-projection columns per core

_CACHE = {}


def _mask_classes(m):
    """m: [sq, sk] fp32. Returns tuple of tuples cls[sb][t] in
    {'skip','zero','add'} for [sq=512, sk=128] blocks."""
    cls = []
    for sb in range(NSB):
        row = []
        sub_sq = m[sb * SBW:(sb + 1) * SBW]
        for t in range(SKT):
            sub = sub_sq[:, t * 128:(t + 1) * 128]
            if np.all(sub < -1e8):
                row.append('skip')
            elif np.all(sub == 0.0):
                row.append('zero')
            else:
                row.append('add')
        cls.append(tuple(row))
    return tuple(cls)


def _build(mask_cls):
    import sys
    if '/opt/trn_rl_repo' not in sys.path:
        sys.path.insert(0, '/opt/trn_rl_repo')
    import concourse.bass as bass  # noqa: F401
    import concourse.mybir as mybir
    import concourse.tile as tile
    from concourse import bacc
    from concourse.masks import make_identity

    f32 = mybir.dt.float32
    bf16 = mybir.dt.bfloat16
    AF = mybir.ActivationFunctionType
    ALU = mybir.AluOpType
    IDM = list(range(32))  # identity lane mask for stream_shuffle

    nc = bacc.Bacc("TRN2", target_bir_lowering=False, debug=False,
                   enable_asserts=False, num_devices=NCORES)

    xT = nc.dram_tensor("xT", [DIM, S], bf16, kind="ExternalInput").ap()
    wq = nc.dram_tensor("wqkvT", [DIM, E_TILES * 128], bf16,
                        kind="ExternalInput").ap()
    woT = nc.dram_tensor("woT", [DIM, EC], bf16, kind="ExternalInput").ap()
    maskT = nc.dram_tensor("maskT", [S, S], bf16, kind="ExternalInput").ap()
    cosq = nc.dram_tensor("cosq", [HD, S], f32, kind="ExternalInput").ap()
    sinq = nc.dram_tensor("sinq", [HD, S], f32, kind="ExternalInput").ap()
    cosk = nc.dram_tensor("cosk", [HD, S], f32, kind="ExternalInput").ap()
    sink = nc.dram_tensor("sink", [HD, S], f32, kind="ExternalInput").ap()
    out = nc.dram_tensor("out", [S, EC], f32, kind="ExternalOutput").ap()

    with tile.TileContext(nc) as tc:
        with (
            tc.tile_pool(name="const", bufs=1) as cp,
            tc.tile_pool(name="pers", bufs=1) as pers,
            tc.tile_pool(name="dram", bufs=1, space="DRAM") as dram,
        ):
            ident = cp.tile([128, 128], bf16, name="ident")
            make_identity(nc, ident)
            ones_d = cp.tile([128, 1], f32, name="ones_d")
            nc.vector.memset(ones_d, 1.0)
            ones_r = cp.tile([1, 128], f32, name="ones_r")
            nc.vector.memset(ones_r, 1.0)

            qT = pers.tile([128, QH * S], bf16, name="qT")
            kT = pers.tile([128, S], bf16, name="kT")
            vv = pers.tile([128, SKT * HD], bf16, name="vv")
            y_bounce = dram.tile([QH * HD, S], bf16, name="y_bounce")
            y_all = dram.tile([DIM, S], bf16, name="y_all",
                              addr_space="Shared")

            # ---------------- stage 1: fused QKV projection + RoPE ------
            with (
                tc.tile_pool(name="s1", bufs=1) as s1,
                tc.tile_pool(name="s1ps", bufs=1, space="PSUM") as s1ps,
            ):
                ctab = {}
                for nm, src in (("cq", cosq), ("sq", sinq),
                                ("ck", cosk), ("sk", sink)):
                    t_ = s1.tile([HD, S], f32, name=f"tab_{nm}", tag=f"tab_{nm}")
                    nc.sync.dma_start(out=t_, in_=src)
                    ctab[nm] = t_

                wts = []
                for k in range(KT):
                    wt = s1.tile([128, E_TILES * 128], bf16, name=f"w1_{k}",
                                 tag=f"w1_{k}")
                    nc.sync.dma_start(out=wt, in_=wq[k * 128:(k + 1) * 128, :])
                    wts.append(wt)

                def rope(dst, ps, cos_t, sin_t, sb):
                    sl = slice(sb * SBW, (sb + 1) * SBW)
                    qf = s1.tile([128, SBW], f32, name="r_qf", tag="r_qf",
                                 bufs=3)
                    nc.vector.tensor_copy(qf, ps)
                    qs = s1.tile([128, SBW], f32, name="r_qs", tag="r_qs",
                                 bufs=3)
                    for (a, b) in ((2, 0), (3, 1), (0, 2), (1, 3)):
                        nc.vector.stream_shuffle(
                            qs[b * 32:(b + 1) * 32, :],
                            qf[a * 32:(a + 1) * 32, :], IDM)
                    t1 = s1.tile([128, SBW], f32, name="r_t1", tag="r_t1",
                                 bufs=3)
                    nc.vector.tensor_tensor(t1, qf, cos_t[:, sl], ALU.mult)
                    t2 = s1.tile([128, SBW], f32, name="r_t2", tag="r_t2",
                                 bufs=3)
                    nc.vector.tensor_tensor(t2, qs, sin_t[:, sl], ALU.mult)
                    nc.vector.tensor_tensor(dst, t1, t2, ALU.add)

                for sb in range(NSB):
                    ps = [s1ps.tile([128, SBW], f32, name=f"qkv_ps{e}",
                                    tag=f"ps{e}") for e in range(E_TILES)]
                    for k in range(KT):
                        xt = s1.tile([128, SBW], bf16, name="xt", tag="xt",
                                     bufs=4)
                        nc.sync.dma_start(
                            out=xt,
                            in_=xT[k * 128:(k + 1) * 128,
                                   sb * SBW:(sb + 1) * SBW])
                        for e in range(E_TILES):
                            nc.tensor.matmul(
                                ps[e], wts[k][:, e * 128:(e + 1) * 128], xt,
                                start=(k == 0), stop=(k == KT - 1))
                    for h in range(QH):
                        dst = qT[:, h * S + sb * SBW: h * S + (sb + 1) * SBW]
                        rope(dst, ps[h], ctab["cq"], ctab["sq"], sb)
                    rope(kT[:, sb * SBW:(sb + 1) * SBW], ps[4],
                         ctab["ck"], ctab["sk"], sb)
                    # v: convert + transpose to [sk, hd] tiles
                    vt = s1.tile([128, SBW], bf16, name="vt", tag="vt", bufs=2)
                    nc.vector.tensor_copy(vt, ps[5])
                    for i in range(4):
                        vp = s1ps.tile([128, 128], bf16, name="v_ps",
                                       tag="vps", bufs=2)
                        nc.tensor.transpose(vp, vt[:, i * 128:(i + 1) * 128],
                                            ident)
                        skt = sb * 4 + i
                        nc.vector.tensor_copy(
                            vv[:, skt * HD:(skt + 1) * HD], vp)

            # ---------------- stage 2: attention -----------------------
            with (
                tc.tile_pool(name="at", bufs=1) as at,
                tc.tile_pool(name="atps", bufs=1, space="PSUM") as atps,
            ):
                for sb in range(NSB):
                    mts = {}
                    for t in range(SKT):
                        if mask_cls[sb][t] != 'add':
                            continue
                        mt = at.tile([128, SBW], bf16, name=f"mt{t}",
                                     tag=f"mt{t % 8}", bufs=2)
                        nc.sync.dma_start(
                            out=mt,
                            in_=maskT[t * 128:(t + 1) * 128,
                                      sb * SBW:(sb + 1) * SBW])
                        mts[t] = mt
                    live = [t for t in range(SKT) if mask_cls[sb][t] != 'skip']
                    for h in range(QH):
                        qsl = qT[:, h * S + sb * SBW: h * S + (sb + 1) * SBW]
                        yacc = atps.tile([128, SBW], f32, name="yacc",
                                         tag="yacc", bufs=2)
                        dacc = at.tile([128, SBW], f32, name="dacc",
                                       tag="dacc", bufs=2)
                        for j, t in enumerate(live):
                            sps = atps.tile([128, SBW], f32, name="sps",
                                            tag="sps", bufs=2)
                            nc.tensor.matmul(
                                sps, kT[:, t * 128:(t + 1) * 128], qsl,
                                start=True, stop=True)
                            if t in mts:
                                nc.vector.tensor_tensor(sps, sps, mts[t],
                                                        ALU.add)
                            pt = at.tile([128, SBW], bf16, name="pt",
                                         tag="pt", bufs=3)
                            nc.scalar.activation(pt, sps, AF.Exp)
                            nc.tensor.matmul(
                                yacc, vv[:, t * HD:(t + 1) * HD], pt,
                                start=(j == 0), stop=(j == len(live) - 1))
                            if j == 0:
                                nc.vector.tensor_copy(dacc, pt)
                            else:
                                nc.vector.tensor_tensor(dacc, dacc, pt,
                                                        ALU.add)
                        dps = atps.tile([1, SBW], f32, name="dps", tag="dps",
                                        bufs=2)
                        nc.tensor.matmul(dps, ones_d, dacc,
                                         start=True, stop=True)
                        rec = at.tile([1, SBW], f32, name="rec", tag="rec",
                                      bufs=2)
                        nc.vector.reciprocal(rec, dps)
                        bps = atps.tile([128, SBW], f32, name="bps",
                                        tag="bps", bufs=2)
                        nc.tensor.matmul(bps, ones_r, rec,
                                         start=True, stop=True)
                        bsb = at.tile([128, SBW], f32, name="bsb", tag="bsb",
                                      bufs=2)
                        nc.vector.tensor_copy(bsb, bps)
                        yn = at.tile([128, SBW], bf16, name="yn", tag="yn",
                                     bufs=2)
                        nc.vector.tensor_tensor(yn, yacc, bsb, ALU.mult)
                        nc.sync.dma_start(
                            out=y_bounce[h * HD:(h + 1) * HD,
                                         sb * SBW:(sb + 1) * SBW],
                            in_=yn)

            # ---------------- AllGather of y^T --------------------------
            nc.gpsimd.collective_compute(
                "AllGather",
                mybir.AluOpType.bypass,
                replica_groups=[list(range(NCORES))],
                ins=[y_bounce.opt()],
                outs=[y_all.opt()],
            )

            # ---------------- stage 3: output projection ---------------
            with (
                tc.tile_pool(name="s3", bufs=1) as s3,
                tc.tile_pool(name="s3ps", bufs=1, space="PSUM") as s3ps,
            ):
                wo_sb = s3.tile([128, KT * EC], bf16, name="wo_sb")
                wo_r = woT.rearrange("(k p) e -> p k e", p=128)
                for k in range(KT):
                    nc.sync.dma_start(out=wo_sb[:, k * EC:(k + 1) * EC],
                                      in_=wo_r[:, k, :])
                y_r = y_all.rearrange("(k p) s -> p k s", p=128)
                for st in range(S // 128):
                    yt = s3.tile([128, KT * 128], bf16, name="yt", tag="yt",
                                 bufs=3)
                    nc.sync.dma_start(
                        out=yt,
                        in_=y_r[:, :, st * 128:(st + 1) * 128])
                    ops = s3ps.tile([128, EC], f32, name="ops", tag="ops",
                                    bufs=2)
                    for k in range(KT):
                        nc.tensor.matmul(
                            ops, yt[:, k * 128:(k + 1) * 128],
                            wo_sb[:, k * EC:(k + 1) * EC],
                            start=(k == 0), stop=(k == KT - 1))
                    osb = s3.tile([128, EC], f32, name="osb", tag="osb",
                                  bufs=2)
                    nc.scalar.activation(osb, ops, AF.Copy)
                    nc.sync.dma_start(out=out[st * 128:(st + 1) * 128, :],
                                      in_=osb)

    nc.finalize()
    return nc


def _prep_inputs(x, wqkv, wo, mask):
    import ml_dtypes
    bf = ml_dtypes.bfloat16

    x2 = np.ascontiguousarray(np.asarray(x, np.float32).reshape(S, DIM))
    xTh = np.ascontiguousarray(x2.T).astype(bf)

    m = np.asarray(mask, np.float32).reshape(S, S)
    mTh = np.ascontiguousarray(m.T).astype(bf)

    inv = 1.0 / (10000.0 ** (np.arange(0, HD, 2, dtype=np.float32)
                             / np.float32(HD)))
    tpos = np.arange(S, dtype=np.float32)
    freqs = np.outer(tpos, inv)
    emb = np.concatenate([freqs, freqs], axis=1)          # [S, 128]
    cosT = np.ascontiguousarray(np.cos(emb).astype(np.float32).T)  # [128, S]
    sinT = np.ascontiguousarray(np.sin(emb).astype(np.float32).T)
    sinmod = np.concatenate([-sinT[:64], sinT[64:]], axis=0)
    scale = np.float32(1.0 / math.sqrt(HD))
    tabs = dict(
        cosq=np.ascontiguousarray(cosT * scale),
        sinq=np.ascontiguousarray(sinmod * scale),
        cosk=cosT,
        sink=np.ascontiguousarray(sinmod),
    )

    wqkv = np.asarray(wqkv, np.float32)
    wo = np.asarray(wo, np.float32)
    in_maps = []
    for c in range(NCORES):
        wq_c = np.concatenate([
            wqkv[512 * c:512 * (c + 1)],                 # 4 q heads
            wqkv[4096 + 128 * c:4096 + 128 * (c + 1)],   # kv head c: k
            wqkv[5120 + 128 * c:5120 + 128 * (c + 1)],   # kv head c: v
        ], axis=0)                                        # [768, 4096]
        wq_cT = np.ascontiguousarray(wq_c.T).astype(bf)   # [4096, 768]
        wo_cT = np.ascontiguousarray(wo[EC * c:EC * (c + 1)].T).astype(bf)
        in_maps.append({
            "xT": xTh, "wqkvT": wq_cT, "woT": wo_cT, "maskT": mTh, **tabs,
        })
    return in_maps, m


def kernel(x, wqkv, wo, mask):
    import sys
    if '/opt/trn_rl_repo' not in sys.path:
        sys.path.insert(0, '/opt/trn_rl_repo')
    from concourse.bass_utils import run_bass_kernel_spmd

    in_maps, m = _prep_inputs(x, wqkv, wo, mask)
    key = _mask_classes(m)
    if key not in _CACHE:
        _CACHE[key] = _build(key)
    nc = _CACHE[key]

    res = run_bass_kernel_spmd(nc, in_maps, list(range(NCORES))).results
    outc = np.concatenate([np.asarray(res[c]["out"]) for c in range(NCORES)],
                          axis=1)
    return outc.reshape(1, S, DIM).astype(np.float32)
